# revision 54
# baseline (speedup 1.0000x reference)
"""Distributed multi-head attention (L=4096, D=2048, H=16, d=128) on 8 TRN2 cores.

Strategy: tensor-parallel over heads (2 heads per core) for QKV projections +
attention, then AllToAll (4x1024-row pieces, overlapped with the attention
stream) to switch to sequence-parallel for the output projection. Each core
returns 512 rows of the final output; the host reassembles.

Per-core dataflow (matmuls in bf16, f32 PSUM accumulation):
  1. QT/KT = Wq/Wk_shard.T @ x.T   ([d, L] layout, head dim on partitions)
     V     = x @ Wv_shard          ([L, f] layout, ones columns interleaved)
     RoPE applied to QT/KT in [d, L] layout with host-prepared C/S tables
     (the 1/sqrt(d) scale is folded into the exp activation).
  2. Per head: scoresT[k, q] = KT_tile.T @ QT_block -> exp on ScalarE -> PT
     bf16; out[q, d|sum] = PT.T @ [V|1] (softmax denominator rides along as a
     129th matmul column); rows normalized by its reciprocal, then transposed
     on the TensorE so the AllToAll lands in [d_concat, q] (lhsT) layout.
  3. After each A2A piece the core holds AT_s[2048, q_s] for its own output
     rows; plain chunk loads feed out = A @ Wo.

Key optimizations over the v1 baseline (trace-driven; PE measured at
~2.0 GHz sustained = the P0 power state, so matmul floors are N/2.0GHz):
- scores/exp tiles [128,1536] (3 PSUM banks, 2 buffers): 11 ScalarE exp
  instructions per window; three 129-wide softmax accumulators pack into
  one PSUM bank (av_bank) to keep the total at 8 banks.
- startup: 8-piece weight loads + 32 half-size xt tiles ordered by
  first-use; PE warm-up matmuls run on a vector-memset tile so they are
  not gated on any HBM load (engine memsets must not sit at the head of
  a DMA-issuing queue).
- 4x1024-row A2A splits: comm(s) at window 4s+4, at-load 2 windows ahead
  of the first Wo chunk (engine semaphores are monotonic counters, so a
  matmul emitted after a slow DMA on its wait-engine stalls on it), one
  Wo chunk per window from w=8.
- epilogue (reciprocal+mul on DVE) deferred past the next scores tile so
  the following AV group's start-matmul never waits on it; Wo out-writes
  ride the gpsimd queue so they cannot FIFO-block epilogue transposes on
  the sync queue ahead of the final collective.
- AV interleave chunked (12,12,8) per 32-matmul accumulation group so
  group boundaries land next to scores tiles.
"""

import os
import sys
import types

import numpy as np
import ml_dtypes

import concourse.bass as bass
import concourse.mybir as mybir
import concourse.tile as tile
from concourse import bacc
from concourse.bass_utils import run_bass_kernel_spmd
from concourse.tile_rust import add_dep_helper

BF16 = mybir.dt.bfloat16
F32 = mybir.dt.float32
nbf16 = ml_dtypes.bfloat16

N_CORES = 8
L = 4096
D = 2048
HPC = 2  # heads per core
HD = 128  # head dim
FC = HPC * HD  # 256: per-core projection width
KCH = D // 128  # 16 contraction chunks
SCALE = 1.0 / float(np.sqrt(HD))
QB = 512  # attention q block
NKK = L // 128  # 32 key tiles

# A2A splits: (q0, qlen, block_q). Output rows of core c:
#   s -> global [s*1024 + c*128, +128)
SPLITS = [(0, 1024, 128), (1024, 1024, 128), (2048, 1024, 128), (3072, 1024, 128)]
OUT_ROW0 = [0, 128, 256, 384]

# k-tiles per scores/exp tile: 16 tiles of 2 k-tiles cover a window
TK = [2] * 16

# module-level knobs (test.py pokes these)
TRACE = False
LAST_RESULTS = None
_CACHED = {}


def _patch_walrus_flags():
    from concourse import bass_utils as _bu

    if getattr(_bu, "_ldw_patched", False):
        return
    _orig = _bu.run_command

    def _patched(cmd, **kw):
        cmd = [c for c in cmd]
        return _orig(cmd, **kw)

    _bu.run_command = _patched
    _bu._ldw_patched = True


def _install_ntff_hook():
    """Enable NTFF profiling under axon (the container lacks antenv.axon_hooks)."""
    try:
        if "antenv.axon_hooks" not in sys.modules:
            mod = types.ModuleType("antenv.axon_hooks")
            _hook = [None]
            mod.set_axon_ntff_profile_hook = lambda h: _hook.__setitem__(0, h)
            mod.get_axon_ntff_profile_hook = lambda: _hook[0]
            sys.modules["antenv.axon_hooks"] = mod
            import antenv

            antenv.axon_hooks = mod
        from antenv.axon_hooks import set_axon_ntff_profile_hook
        from trn_agent_boot.trn_boot import _ntff_profile_via_ctypes

        set_axon_ntff_profile_hook(_ntff_profile_via_ctypes("/opt/axon/libaxon_pjrt.so"))
        from concourse import bass_utils

        bass_utils.upload_artifacts = lambda tmpdir: tmpdir
    except Exception:
        pass


def build_nc():
    nc = bacc.Bacc(None, target_bir_lowering=False, num_devices=N_CORES)

    xT_ext = nc.declare_dram_parameter("xT", [D, L], BF16, isOutput=False)
    wq_ext = nc.declare_dram_parameter("wq", [D, FC], BF16, isOutput=False)
    wk_ext = nc.declare_dram_parameter("wk", [D, FC], BF16, isOutput=False)
    wv_ext = nc.declare_dram_parameter("wv", [D, FC], BF16, isOutput=False)
    wo_ext = nc.declare_dram_parameter("wo", [D, D], BF16, isOutput=False)
    ctab_ext = nc.declare_dram_parameter("ctab", [128, L], BF16, isOutput=False)
    stab_ext = nc.declare_dram_parameter("stab", [128, L], BF16, isOutput=False)
    ident_ext = nc.declare_dram_parameter("ident", [128, 128], F32, isOutput=False)
    out_ext = nc.declare_dram_parameter("out", [512, D], F32, isOutput=True)

    # A2A bounces in [d_concat-block, q] layout: rows = 8 blocks x (2 heads x 128 d)
    a2a_in = [
        nc.dram_tensor(f"a2a_in{s}", [8 * FC, bq], BF16)
        for s, (_, _, bq) in enumerate(SPLITS)
    ]
    a2a_out = [
        nc.dram_tensor(f"a2a_out{s}", [8 * FC, bq], BF16)
        for s, (_, _, bq) in enumerate(SPLITS)
    ]

    sync_in = nc.dram_tensor("sync_in", [8, 64], BF16)
    sync_out = nc.dram_tensor("sync_out", [8, 64], BF16)

    with tile.TileContext(nc) as tc:
        with tc.tile_pool(name="persist", bufs=1) as persist:
            # barrier-warming collective: absorbs per-core start skew so the
            # first real AllToAll doesn't pay it
            nc.gpsimd.collective_compute(
                "AllToAll",
                mybir.AluOpType.bypass,
                replica_groups=[list(range(N_CORES))],
                ins=[sync_in.ap().opt()],
                outs=[sync_out.ap().opt()],
            )
            # persistent tiles (no DMA yet; emission order sets DMA priority)
            qt = persist.tile([128, HPC * L], BF16, tag="qt")
            kt = persist.tile([128, HPC * L], BF16, tag="kt")
            va = persist.tile([128, (L // 128) * 260], BF16, tag="va")
            wo_sb = persist.tile([128, KCH * D], BF16, tag="wo")
            warm = persist.tile([128, 8], BF16, tag="warm")
            ident_sb = persist.tile([128, 128], F32, tag="ident")
            nc.sync.dma_start(ident_sb[:], ident_ext[:])

            QL = 1024  # L columns per xT load round
            with (
                tc.tile_pool(name="p1sb", bufs=1) as p1,
                tc.tile_pool(name="p1ps", bufs=1, space="PSUM") as p1ps,
            ):
                # critical-path loads first: wq + the 16 first-half xt tiles
                # (everything the first Q projection block needs), then the
                # rest ordered by first-use time.
                ctab = p1.tile([128, L], BF16, tag="ctab")
                stab = p1.tile([128, L], BF16, tag="stab")
                wq_sb = p1.tile([128, KCH * FC], BF16, tag="wq")
                wk_sb = p1.tile([128, KCH * FC], BF16, tag="wk")
                wv_sb = p1.tile([128, KCH * FC], BF16, tag="wv")
                xt_q0_lb0 = []
                for jp in range(8):
                    nc.gpsimd.dma_start(
                        wq_sb[:, jp * 2 * FC : (jp + 1) * 2 * FC].rearrange(
                            "p (k f) -> p k f", k=2
                        ),
                        wq_ext[jp * 256 : (jp + 1) * 256, :].rearrange(
                            "(k p) f -> p k f", p=128
                        ),
                    )
                    for kc in (2 * jp, 2 * jp + 1):
                        xt_t = p1.tile([128, 512], BF16, tag="xt", bufs=36)
                        nc.sync.dma_start(
                            xt_t[:], xT_ext[kc * 128 : (kc + 1) * 128, 0:512]
                        )
                        xt_q0_lb0.append(xt_t)

                def load_xt_halves(l0, lb, all_sync=False):
                    ts = []
                    for kc in range(KCH):
                        xt_t = p1.tile([128, 512], BF16, tag="xt", bufs=36)
                        eng = nc.sync if all_sync else (nc.gpsimd, nc.sync)[kc % 2]
                        eng.dma_start(
                            xt_t[:],
                            xT_ext[
                                kc * 128 : (kc + 1) * 128,
                                l0 + lb * 512 : l0 + (lb + 1) * 512,
                            ],
                        )
                        ts.append(xt_t)
                    return ts

                # PE warm-up: dummy matmuls on a tiny vector-memset tile keep
                # the HAM activity window tripped during the initial DMA wait,
                # so the real matmul stream starts at full clock. The memset
                # rides the idle vector engine: an engine-op at the head of
                # the gpsimd queue would delay every startup DMA behind it.
                wtile = p1.tile([128, 128], BF16, tag="wt")
                nc.vector.memset(wtile[:], 1.0)
                wmt = p1ps.tile([128, 512], F32, tag="pj", bufs=4)
                for _ in range(110):
                    nc.tensor.matmul(wmt[0:64, 0:64], wtile[:, 0:64], wtile[:, 64:128])
                xts_q0 = (xt_q0_lb0, load_xt_halves(0, 1))
                nc.gpsimd.dma_start(ctab[:], ctab_ext[:])
                nc.gpsimd.dma_start(stab[:], stab_ext[:])
                for jp in range(4):
                    nc.gpsimd.dma_start(
                        wk_sb[:, jp * 4 * FC : (jp + 1) * 4 * FC].rearrange(
                            "p (k f) -> p k f", k=4
                        ),
                        wk_ext[jp * 512 : (jp + 1) * 512, :].rearrange(
                            "(k p) f -> p k f", p=128
                        ),
                    )
                # va memset sits after the critical loads on the gpsimd queue
                # (the engine-op takes ~7us; va isn't needed until ~45us)
                nc.gpsimd.memset(va[:], 1.0)
                for jp in range(4):
                    nc.gpsimd.dma_start(
                        wv_sb[:, jp * 4 * FC : (jp + 1) * 4 * FC].rearrange(
                            "p (k f) -> p k f", k=4
                        ),
                        wv_ext[jp * 512 : (jp + 1) * 512, :].rearrange(
                            "(k p) f -> p k f", p=128
                        ),
                    )
                nc.scalar.activation(
                    warm[:], ctab[:, 0:8], mybir.ActivationFunctionType.Exp
                )
                # 4 pieces: a single-instruction DMA runs on one engine at
                # ~22.5GB/s (8MB would take ~350us); pieces transfer in parallel
                for jp in range(4):
                    nc.gpsimd.dma_start(
                        wo_sb[:, jp * 4 * D : (jp + 1) * 4 * D].rearrange(
                            "p (k f) -> p k f", k=4
                        ),
                        wo_ext[jp * 512 : (jp + 1) * 512, :].rearrange(
                            "(k p) f -> p k f", p=128
                        ),
                    )

                # ---------------- Phase 1: QKV projections + RoPE ------------
                # xts holds 32 half tiles [128,512]: index kc*2 + lb
                for quarter in range(L // QL):
                    l0 = quarter * QL
                    if quarter == 0:
                        xts = xts_q0
                    else:
                        xts = (load_xt_halves(l0, 0), load_xt_halves(l0, 1))
                    # Q and K projections (transposed layout) + rope
                    for (w_sb, dst) in ((wq_sb, qt), (wk_sb, kt)):
                        for h in range(HPC):
                            for lb in range(QL // 512):
                                ps = p1ps.tile([128, 512], F32, tag="pj", bufs=4)
                                for kc in range(KCH):
                                    nc.tensor.matmul(
                                        ps[:],
                                        w_sb[:, kc * FC + h * HD : kc * FC + (h + 1) * HD],
                                        xts[lb][kc][:],
                                        start=(kc == 0),
                                        stop=(kc == KCH - 1),
                                    )
                                lsl = slice(l0 + lb * 512, l0 + (lb + 1) * 512)
                                tmp = p1.tile([128, 512], BF16, tag="tmp", bufs=4)
                                nc.scalar.copy(tmp[:], ps[:])
                                rot = p1.tile([128, 512], BF16, tag="rot", bufs=4)
                                for (a, b) in ((0, 64), (32, 96), (64, 0), (96, 32)):
                                    nc.vector.tensor_copy(
                                        rot[a : a + 32, :], tmp[b : b + 32, :]
                                    )
                                t1 = p1.tile([128, 512], BF16, tag="t1", bufs=4)
                                nc.vector.tensor_mul(t1[:], tmp[:], ctab[:, lsl])
                                t2 = p1.tile([128, 512], BF16, tag="t2", bufs=4)
                                nc.vector.tensor_mul(t2[:], rot[:], stab[:, lsl])
                                dsl = slice(
                                    h * L + l0 + lb * 512, h * L + l0 + (lb + 1) * 512
                                )
                                nc.vector.tensor_add(dst[:, dsl], t1[:], t2[:])
                    # V projection (natural layout), strided copy into va
                    for lt in range(QL // 128):
                        psv = p1ps.tile([128, FC], F32, tag="pv", bufs=3)
                        for kc in range(KCH):
                            nc.tensor.matmul(
                                psv[:],
                                xts[lt // 4][kc][:, (lt % 4) * 128 : (lt % 4 + 1) * 128],
                                wv_sb[:, kc * FC : (kc + 1) * FC],
                                start=(kc == 0),
                                stop=(kc == KCH - 1),
                            )
                        gt = quarter * (QL // 128) + lt  # global L tile 0..31
                        dst = va[:, gt * 260 : (gt + 1) * 260].rearrange(
                            "p (g j) -> p g j", g=2
                        )[:, :, 0:128]
                        nc.vector.tensor_copy(
                            dst, psv[:].rearrange("p (g j) -> p g j", g=2)
                        )

            # -------- Phase 2: attention windows + overlapped A2A/Wo ---------
            windows = [(qb, h) for qb in range(L // QB) for h in range(HPC)]
            with (
                tc.tile_pool(name="p2sb", bufs=1) as p2,
                tc.tile_pool(name="p2ps", bufs=1, space="PSUM") as p2ps,
                tc.tile_pool(name="p3sb", bufs=1) as p3,
                tc.tile_pool(name="p3ps", bufs=1, space="PSUM") as p3ps,
            ):
                pt_store = {}
                # one PSUM bank holds three packed 129-wide softmax accumulators
                av_bank = p2ps.tile([128, 512], F32, tag="avb", bufs=1)
                tr_bank = p2ps.tile([128, 512], F32, tag="tr", bufs=1)
                av_slot = [0]
                pending_epi = []

                def emit_epilogue(w, qs, base):
                    qb, h = windows[w]
                    rec = p2.tile([128, 1], F32, tag="rec", bufs=8)
                    nc.vector.reciprocal(rec[:], av_bank[:, base + 128 : base + 129])
                    osb = p2.tile([128, 128], F32, tag="osb", bufs=8)
                    nc.vector.tensor_scalar_mul(
                        osb[:], av_bank[:, base : base + 128], rec[:]
                    )
                    # PE transpose (short matmul) instead of the xbar DMA
                    # transpose (a globally serialized ~4.9us resource)
                    trs = tr_bank[:, qs * 128 : (qs + 1) * 128]
                    nc.tensor.transpose(trs, osb[:], ident_sb[:])
                    ot = p2.tile([128, 128], BF16, tag="ot", bufs=12)
                    nc.vector.tensor_copy(ot[:], trs)
                    t = qb * (QB // 128) + qs  # global q tile 0..31
                    qg = t * 128
                    s = next(
                        i for i, (q0, ql, _) in enumerate(SPLITS) if q0 <= qg < q0 + ql
                    )
                    q0, _, bq = SPLITS[s]
                    tt = (qg - q0) // 128
                    tpb = bq // 128
                    j, co = tt // tpb, (tt % tpb) * 128
                    nc.gpsimd.dma_start(
                        a2a_in[s][j * FC + h * HD : j * FC + (h + 1) * HD, co : co + 128],
                        ot[:],
                    )

                def flush_epilogues():
                    while pending_epi:
                        emit_epilogue(*pending_epi.pop(0))

                def emit_score_tile(w, kk0, tk):
                    qb, h = windows[w]
                    sc = p2ps.tile([128, 1024], F32, tag="sc", bufs=2)
                    for jt in range(tk):
                        kk = kk0 + jt
                        nc.tensor.matmul(
                            sc[:, jt * 512 : (jt + 1) * 512],
                            kt[:, h * L + kk * 128 : h * L + (kk + 1) * 128],
                            qt[:, h * L + qb * QB : h * L + (qb + 1) * QB],
                        )
                    pt = p2.tile([128, 1024], BF16, tag="pt", bufs=32)
                    nc.scalar.activation(
                        pt[:],
                        sc[:],
                        mybir.ActivationFunctionType.Exp,
                        scale=SCALE,
                    )
                    return pt

                def emit_window(w):
                    # scores+exp for window w interleaved with AV for w-1, so
                    # the PE never idles waiting on the exp pipeline
                    prev = pt_store.pop(w - 1, None)
                    if prev is not None:
                        pqb, ph = windows[w - 1]
                    pts = []
                    last_av_mm = None
                    cur = [0, 0]  # [mm cursor, current av slot base]

                    def emit_av(n):
                        nonlocal last_av_mm
                        for _ in range(n):
                            c = cur[0]
                            if c >= 128:
                                return
                            qs, kk = c // 32, c % 32
                            if kk == 0:
                                # a start-MM must never be emitted while the
                                # epilogue that reads its slot is still
                                # pending (WAR would invert into a race)
                                flush_epilogues()
                                cur[1] = (av_slot[0] % 3) * 129
                                av_slot[0] += 1
                            base = cur[1]
                            last_av_mm = nc.tensor.matmul(
                                av_bank[:, base : base + 129],
                                prev[kk // 2][
                                    :,
                                    (kk % 2) * 512
                                    + qs * 128 : (kk % 2) * 512
                                    + (qs + 1) * 128,
                                ],
                                va[:, kk * 260 + ph * 130 : kk * 260 + ph * 130 + 129],
                                start=(kk == 0),
                                stop=(kk == 31),
                                skip_group_check=True,
                            )
                            if kk == 31:
                                # defer: emitting the epilogue here makes the
                                # next group's start-matmul (emitted right
                                # after) wait ~0.65us for the epilogue's DVE
                                # reads of the shared av bank. Emitted after
                                # the next scores tile instead, the DVE reads
                                # overlap PE work and any framework-inserted
                                # ordering lands on the DVE, not the PE.
                                pending_epi.append((w - 1, qs, base))
                            cur[0] = c + 1

                    # AV chunking [12,12,8] per 32-MM accumulation group keeps
                    # group boundaries next to scores tiles
                    CH = (8,) * 16
                    kk0 = 0
                    for t, tk in enumerate(TK):
                        if w < len(windows):
                            pts.append(emit_score_tile(w, kk0, tk))
                            flush_epilogues()
                        kk0 += tk
                        if prev is not None:
                            emit_av(CH[t])
                    if prev is not None:
                        emit_av(128)  # flush any remainder
                        flush_epilogues()
                    if pts:
                        pt_store[w] = pts
                    return last_av_mm

                ats = {}

                def emit_wo_comm(s):
                    nc.gpsimd.collective_compute(
                        "AllToAll",
                        mybir.AluOpType.bypass,
                        replica_groups=[list(range(N_CORES))],
                        ins=[a2a_in[s].ap().opt()],
                        outs=[a2a_out[s].ap().opt()],
                    )

                def emit_at_load(s, after=None):
                    # deferred so the sync queue isn't head-of-line blocked
                    # on the collective while epilogue transposes queue up.
                    # 4 pieces: one dma_start runs on a single DMA engine at
                    # ~22.5GB/s (1MB = ~44us); pieces transfer in parallel
                    bq = SPLITS[s][2]
                    at = p3.tile([128, KCH * 256], BF16, tag="at", bufs=2)
                    for jp in range(8):
                        ld = nc.sync.dma_start(
                            at[:, jp * 2 * bq : (jp + 1) * 2 * bq].rearrange(
                                "p (k q) -> p k q", k=2
                            ),
                            a2a_out[s][jp * 256 : (jp + 1) * 256, :].rearrange(
                                "(k p) q -> p k q", p=128
                            ),
                        )
                        if after is not None:
                            add_dep_helper(
                                ld.ins,
                                after.ins,
                                sync=False,
                                reason="keep AT load behind the attention window",
                            )
                    ats[s] = at

                wo_out_pending = []

                def flush_wo_writes():
                    while wo_out_pending:
                        ob, r0, fb = wo_out_pending.pop(0)
                        nc.gpsimd.dma_start(
                            out_ext[r0 : r0 + 128, fb * 512 : (fb + 1) * 512], ob[:]
                        )

                def emit_wo_chunk(s, g, after=None, alt=False, defer_out=False):
                    bq = SPLITS[s][2]
                    rt, fb = g // 4, g % 4
                    if alt:
                        # tail only: scores are done, so a dead sc slot serves
                        # as the second po buffer (po itself is single-buffered)
                        po = p2ps.tile([128, 1024], F32, tag="sc", bufs=2)
                    else:
                        po = p3ps.tile([128, 512], F32, tag="po", bufs=1)
                    for kc in range(KCH):
                        mm = nc.tensor.matmul(
                            po[:, 0:512],
                            ats[s][:, kc * bq + rt * 128 : kc * bq + (rt + 1) * 128],
                            wo_sb[:, kc * D + fb * 512 : kc * D + (fb + 1) * 512],
                            start=(kc == 0),
                            stop=(kc == KCH - 1),
                            skip_group_check=True,
                        )
                        if kc == 0 and after is not None:
                            add_dep_helper(
                                mm.ins,
                                after.ins,
                                sync=False,
                                reason="keep Wo chunk behind the attention window",
                            )
                    ob = p3.tile([128, 512], F32, tag="ob", bufs=3)
                    r0 = OUT_ROW0[s] + rt * 128
                    if defer_out:
                        # tail chunks: the out-write trigger would wait on the
                        # ob copy at the gpsimd queue head and FIFO-block the
                        # final epilogue a2a writes + collective trigger.
                        # Deferred until after the comm(3) emission.
                        nc.vector.tensor_copy(ob[:], po[:, 0:512])
                        wo_out_pending.append((ob, r0, fb))
                    else:
                        # two half copies + half writes: the write of half 0
                        # overlaps the copy of half 1, and the two 128KB DMAs
                        # land on separate engines (halves the final drain);
                        # gpsimd queue: a2a writes behind have windows of slack
                        for hh in range(2):
                            nc.vector.tensor_copy(
                                ob[:, hh * 256 : (hh + 1) * 256],
                                po[:, hh * 256 : (hh + 1) * 256],
                            )
                            nc.gpsimd.dma_start(
                                out_ext[
                                    r0 : r0 + 128,
                                    fb * 512 + hh * 256 : fb * 512 + (hh + 1) * 256,
                                ],
                                ob[:, hh * 256 : (hh + 1) * 256],
                            )

                # 4-way splits: each split's epilogues finish at window 4s+4
                # and its collective launches there; the at load runs 2 windows
                # ahead of the first Wo chunk so the chunk's matmuls never
                # head-of-line-block the PE on the load DMA. One chunk per
                # window keeps the PE stream smooth; the final collective is
                # emitted before the split-2 tail chunks so it triggers the
                # moment the last epilogue lands.
                for w in range(len(windows) + 1):
                    la = emit_window(w)
                    if w == 4:
                        emit_wo_comm(0)
                    if w == 6:
                        emit_at_load(0, after=la)
                    if w == 8:
                        emit_wo_comm(1)
                        emit_wo_chunk(0, 0, after=la)
                    if w == 9:
                        emit_wo_chunk(0, 1, after=la)
                    if w == 10:
                        emit_at_load(1, after=la)
                        emit_wo_chunk(0, 2, after=la)
                    if w == 11:
                        emit_wo_chunk(0, 3, after=la)
                    if w == 12:
                        emit_wo_comm(2)
                        emit_wo_chunk(1, 0, after=la)
                    if w == 13:
                        emit_wo_chunk(1, 1, after=la)
                    if w == 14:
                        emit_at_load(2, after=la)
                        emit_wo_chunk(1, 2, after=la)
                    if w == 15:
                        emit_wo_chunk(1, 3, after=la, defer_out=True)
                        emit_wo_chunk(2, 0, after=la, defer_out=True)
                    if w == 16:
                        # split-2 tail chunks BEFORE the comm trigger: matmuls
                        # emitted after a gpsimd collective trigger wait for
                        # the collective's completion (semaphore-count
                        # inflation), which left the PE idle ~9us here. Their
                        # out-writes are deferred so the trigger stays at the
                        # head of the gpsimd queue right behind the final
                        # epilogue a2a writes (the DVE ob copy still runs
                        # inline, keeping the po chain ordered).
                        emit_wo_chunk(2, 1, defer_out=True)
                        emit_wo_chunk(2, 2, alt=True, defer_out=True)
                        emit_wo_chunk(2, 3, defer_out=True)
                        emit_wo_comm(3)
                flush_wo_writes()
                # at_load(3) emitted only after every matmul that must NOT
                # wait on it: engine semaphores are monotonic counters, so a
                # chunk emitted after this load would wait for the load's
                # completion count (observed as a 44us PE stall)
                emit_at_load(3)
                emit_wo_chunk(3, 0, alt=True)
                emit_wo_chunk(3, 1)
                emit_wo_chunk(3, 2, alt=True)
                emit_wo_chunk(3, 3)

    nc.compile()
    return nc


def _host_prep(x, Wq, Wk, Wv, Wo, sin, cos):
    xT = np.ascontiguousarray(x.T).astype(nbf16)
    wo_b = np.ascontiguousarray(Wo).astype(nbf16)
    c64 = cos.reshape(L, 64)
    s64 = sin.reshape(L, 64)
    ctab = np.ascontiguousarray(np.concatenate([c64, c64], axis=1).T).astype(nbf16)
    stab = np.ascontiguousarray(np.concatenate([-s64, s64], axis=1).T).astype(nbf16)
    ident = np.eye(128, dtype=np.float32)
    in_maps = []
    for c in range(N_CORES):
        sl = slice(c * FC, (c + 1) * FC)
        in_maps.append(
            {
                "xT": xT,
                "wq": np.ascontiguousarray(Wq[:, sl]).astype(nbf16),
                "wk": np.ascontiguousarray(Wk[:, sl]).astype(nbf16),
                "wv": np.ascontiguousarray(Wv[:, sl]).astype(nbf16),
                "wo": wo_b,
                "ctab": ctab,
                "stab": stab,
                "ident": ident,
            }
        )
    return in_maps


def kernel(x, Wq, Wk, Wv, Wo, sin, cos):
    global LAST_RESULTS
    x, Wq, Wk, Wv, Wo = (np.asarray(a, np.float32) for a in (x, Wq, Wk, Wv, Wo))
    sin, cos = np.asarray(sin, np.float32), np.asarray(cos, np.float32)

    _patch_walrus_flags()
    if TRACE:
        _install_ntff_hook()
        os.environ["BASS_TRACE"] = "1"

    if "nc" not in _CACHED:
        _CACHED["nc"] = build_nc()
    nc = _CACHED["nc"]

    in_maps = _host_prep(x, Wq, Wk, Wv, Wo, sin, cos)
    trace_cores = list(range(N_CORES)) if os.environ.get("ALL_CORES") else None
    res = run_bass_kernel_spmd(
        nc, in_maps, core_ids=list(range(N_CORES)), trace=TRACE, trace_cores=trace_cores
    )
    LAST_RESULTS = res

    out = np.empty((L, D), np.float32)
    for c in range(N_CORES):
        oc = res.results[c]["out"]
        for s in range(4):
            out[s * 1024 + c * 128 : s * 1024 + (c + 1) * 128] = oc[
                s * 128 : (s + 1) * 128
            ]
    return out



# revision 56
# speedup vs baseline: 1.0052x; 1.0052x over previous
"""Distributed multi-head attention (L=4096, D=2048, H=16, d=128) on 8 TRN2 cores.

Strategy: tensor-parallel over heads (2 heads per core) for QKV projections +
attention, then AllToAll (4x1024-row pieces, overlapped with the attention
stream) to switch to sequence-parallel for the output projection. Each core
returns 512 rows of the final output; the host reassembles.

Per-core dataflow (matmuls in bf16, f32 PSUM accumulation):
  1. QT/KT = Wq/Wk_shard.T @ x.T   ([d, L] layout, head dim on partitions)
     V     = x @ Wv_shard          ([L, f] layout, ones columns interleaved)
     RoPE applied to QT/KT in [d, L] layout with host-prepared C/S tables
     (the 1/sqrt(d) scale is folded into the exp activation).
  2. Per head: scoresT[k, q] = KT_tile.T @ QT_block -> exp on ScalarE -> PT
     bf16; out[q, d|sum] = PT.T @ [V|1] (softmax denominator rides along as a
     129th matmul column); rows normalized by its reciprocal, then transposed
     on the TensorE so the AllToAll lands in [d_concat, q] (lhsT) layout.
  3. After each A2A piece the core holds AT_s[2048, q_s] for its own output
     rows; plain chunk loads feed out = A @ Wo.

Key optimizations over the v1 baseline (trace-driven; PE measured at
~2.0 GHz sustained = the P0 power state, so matmul floors are N/2.0GHz):
- scores/exp tiles [128,1536] (3 PSUM banks, 2 buffers): 11 ScalarE exp
  instructions per window; three 129-wide softmax accumulators pack into
  one PSUM bank (av_bank) to keep the total at 8 banks.
- startup: 8-piece weight loads + 32 half-size xt tiles ordered by
  first-use; PE warm-up matmuls run on a vector-memset tile so they are
  not gated on any HBM load (engine memsets must not sit at the head of
  a DMA-issuing queue).
- 4x1024-row A2A splits: comm(s) at window 4s+4, at-load 2 windows ahead
  of the first Wo chunk (engine semaphores are monotonic counters, so a
  matmul emitted after a slow DMA on its wait-engine stalls on it), one
  Wo chunk per window from w=8.
- epilogue (reciprocal+mul on DVE) deferred past the next scores tile so
  the following AV group's start-matmul never waits on it; Wo out-writes
  ride the gpsimd queue so they cannot FIFO-block epilogue transposes on
  the sync queue ahead of the final collective.
- AV interleave chunked (12,12,8) per 32-matmul accumulation group so
  group boundaries land next to scores tiles.
"""

import os
import sys
import types

import numpy as np
import ml_dtypes

import concourse.bass as bass
import concourse.mybir as mybir
import concourse.tile as tile
from concourse import bacc
from concourse.bass_utils import run_bass_kernel_spmd
from concourse.tile_rust import add_dep_helper

BF16 = mybir.dt.bfloat16
F32 = mybir.dt.float32
nbf16 = ml_dtypes.bfloat16

N_CORES = 8
L = 4096
D = 2048
HPC = 2  # heads per core
HD = 128  # head dim
FC = HPC * HD  # 256: per-core projection width
KCH = D // 128  # 16 contraction chunks
SCALE = 1.0 / float(np.sqrt(HD))
QB = 512  # attention q block
NKK = L // 128  # 32 key tiles

# A2A splits: (q0, qlen, block_q). Output rows of core c:
#   s -> global [s*1024 + c*128, +128)
SPLITS = [(0, 1024, 128), (1024, 1024, 128), (2048, 1024, 128), (3072, 1024, 128)]
OUT_ROW0 = [0, 128, 256, 384]

# k-tiles per scores/exp tile: 16 tiles of 2 k-tiles cover a window
TK = [2] * 16

# module-level knobs (test.py pokes these)
TRACE = False
LAST_RESULTS = None
_CACHED = {}


def _patch_walrus_flags():
    from concourse import bass_utils as _bu

    if getattr(_bu, "_ldw_patched", False):
        return
    _orig = _bu.run_command

    def _patched(cmd, **kw):
        cmd = [c for c in cmd]
        return _orig(cmd, **kw)

    _bu.run_command = _patched
    _bu._ldw_patched = True


def _install_ntff_hook():
    """Enable NTFF profiling under axon (the container lacks antenv.axon_hooks)."""
    try:
        if "antenv.axon_hooks" not in sys.modules:
            mod = types.ModuleType("antenv.axon_hooks")
            _hook = [None]
            mod.set_axon_ntff_profile_hook = lambda h: _hook.__setitem__(0, h)
            mod.get_axon_ntff_profile_hook = lambda: _hook[0]
            sys.modules["antenv.axon_hooks"] = mod
            import antenv

            antenv.axon_hooks = mod
        from antenv.axon_hooks import set_axon_ntff_profile_hook
        from trn_agent_boot.trn_boot import _ntff_profile_via_ctypes

        set_axon_ntff_profile_hook(_ntff_profile_via_ctypes("/opt/axon/libaxon_pjrt.so"))
        from concourse import bass_utils

        bass_utils.upload_artifacts = lambda tmpdir: tmpdir
    except Exception:
        pass


def build_nc():
    nc = bacc.Bacc(None, target_bir_lowering=False, num_devices=N_CORES)

    xT_ext = nc.declare_dram_parameter("xT", [D, L], BF16, isOutput=False)
    wq_ext = nc.declare_dram_parameter("wq", [D, FC], BF16, isOutput=False)
    wk_ext = nc.declare_dram_parameter("wk", [D, FC], BF16, isOutput=False)
    wv_ext = nc.declare_dram_parameter("wv", [D, FC], BF16, isOutput=False)
    wo_ext = nc.declare_dram_parameter("wo", [D, D], BF16, isOutput=False)
    ctab_ext = nc.declare_dram_parameter("ctab", [128, L], BF16, isOutput=False)
    stab_ext = nc.declare_dram_parameter("stab", [128, L], BF16, isOutput=False)
    ident_ext = nc.declare_dram_parameter("ident", [128, 128], F32, isOutput=False)
    out_ext = nc.declare_dram_parameter("out", [512, D], F32, isOutput=True)

    # A2A bounces in [d_concat-block, q] layout: rows = 8 blocks x (2 heads x 128 d)
    a2a_in = [
        nc.dram_tensor(f"a2a_in{s}", [8 * FC, bq], BF16)
        for s, (_, _, bq) in enumerate(SPLITS)
    ]
    a2a_out = [
        nc.dram_tensor(f"a2a_out{s}", [8 * FC, bq], BF16)
        for s, (_, _, bq) in enumerate(SPLITS)
    ]

    sync_in = nc.dram_tensor("sync_in", [8, 64], BF16)
    sync_out = nc.dram_tensor("sync_out", [8, 64], BF16)

    with tile.TileContext(nc) as tc:
        with tc.tile_pool(name="persist", bufs=1) as persist:
            # barrier-warming collective: absorbs per-core start skew so the
            # first real AllToAll doesn't pay it
            nc.gpsimd.collective_compute(
                "AllToAll",
                mybir.AluOpType.bypass,
                replica_groups=[list(range(N_CORES))],
                ins=[sync_in.ap().opt()],
                outs=[sync_out.ap().opt()],
            )
            # persistent tiles (no DMA yet; emission order sets DMA priority)
            qt = persist.tile([128, HPC * L], BF16, tag="qt")
            kt = persist.tile([128, HPC * L], BF16, tag="kt")
            va = persist.tile([128, (L // 128) * 260], BF16, tag="va")
            wo_sb = persist.tile([128, KCH * D], BF16, tag="wo")
            warm = persist.tile([128, 8], BF16, tag="warm")
            ident_sb = persist.tile([128, 128], F32, tag="ident")
            nc.sync.dma_start(ident_sb[:], ident_ext[:])

            QL = 1024  # L columns per xT load round
            with (
                tc.tile_pool(name="p1sb", bufs=1) as p1,
                tc.tile_pool(name="p1ps", bufs=1, space="PSUM") as p1ps,
            ):
                # critical-path loads first: wq + the 16 first-half xt tiles
                # (everything the first Q projection block needs), then the
                # rest ordered by first-use time.
                ctab = p1.tile([128, L], BF16, tag="ctab")
                stab = p1.tile([128, L], BF16, tag="stab")
                wq_sb = p1.tile([128, KCH * FC], BF16, tag="wq")
                wk_sb = p1.tile([128, KCH * FC], BF16, tag="wk")
                wv_sb = p1.tile([128, KCH * FC], BF16, tag="wv")
                for jp in range(4):
                    nc.gpsimd.dma_start(
                        wq_sb[:, jp * 4 * FC : (jp + 1) * 4 * FC].rearrange(
                            "p (k f) -> p k f", k=4
                        ),
                        wq_ext[jp * 512 : (jp + 1) * 512, :].rearrange(
                            "(k p) f -> p k f", p=128
                        ),
                    )

                def load_xt_halves(l0, lb, all_sync=False):
                    ts = []
                    for kc in range(KCH):
                        xt_t = p1.tile([128, 512], BF16, tag="xt", bufs=36)
                        eng = nc.sync if all_sync else (nc.gpsimd, nc.sync)[kc % 2]
                        eng.dma_start(
                            xt_t[:],
                            xT_ext[
                                kc * 128 : (kc + 1) * 128,
                                l0 + lb * 512 : l0 + (lb + 1) * 512,
                            ],
                        )
                        ts.append(xt_t)
                    return ts

                # PE warm-up: dummy matmuls on a tiny vector-memset tile keep
                # the HAM activity window tripped during the initial DMA wait,
                # so the real matmul stream starts at full clock. The memset
                # rides the idle vector engine: an engine-op at the head of
                # the gpsimd queue would delay every startup DMA behind it.
                wtile = p1.tile([128, 128], BF16, tag="wt")
                nc.vector.memset(wtile[:], 1.0)
                wmt = p1ps.tile([128, 512], F32, tag="pj", bufs=4)
                for _ in range(110):
                    nc.tensor.matmul(wmt[0:64, 0:64], wtile[:, 0:64], wtile[:, 64:128])
                xts_q0 = (load_xt_halves(0, 0), load_xt_halves(0, 1))
                for jp in range(4):
                    nc.gpsimd.dma_start(
                        ctab[:, jp * 1024 : (jp + 1) * 1024],
                        ctab_ext[:, jp * 1024 : (jp + 1) * 1024],
                    )
                    nc.gpsimd.dma_start(
                        stab[:, jp * 1024 : (jp + 1) * 1024],
                        stab_ext[:, jp * 1024 : (jp + 1) * 1024],
                    )
                for jp in range(4):
                    nc.gpsimd.dma_start(
                        wk_sb[:, jp * 4 * FC : (jp + 1) * 4 * FC].rearrange(
                            "p (k f) -> p k f", k=4
                        ),
                        wk_ext[jp * 512 : (jp + 1) * 512, :].rearrange(
                            "(k p) f -> p k f", p=128
                        ),
                    )
                # va memset sits after the critical loads on the gpsimd queue
                # (the engine-op takes ~7us; va isn't needed until ~45us)
                nc.gpsimd.memset(va[:], 1.0)
                for jp in range(4):
                    nc.gpsimd.dma_start(
                        wv_sb[:, jp * 4 * FC : (jp + 1) * 4 * FC].rearrange(
                            "p (k f) -> p k f", k=4
                        ),
                        wv_ext[jp * 512 : (jp + 1) * 512, :].rearrange(
                            "(k p) f -> p k f", p=128
                        ),
                    )
                nc.scalar.activation(
                    warm[:], ctab[:, 0:8], mybir.ActivationFunctionType.Exp
                )
                # 4 pieces: a single-instruction DMA runs on one engine at
                # ~22.5GB/s (8MB would take ~350us); pieces transfer in parallel
                for jp in range(4):
                    nc.gpsimd.dma_start(
                        wo_sb[:, jp * 4 * D : (jp + 1) * 4 * D].rearrange(
                            "p (k f) -> p k f", k=4
                        ),
                        wo_ext[jp * 512 : (jp + 1) * 512, :].rearrange(
                            "(k p) f -> p k f", p=128
                        ),
                    )

                # ---------------- Phase 1: QKV projections + RoPE ------------
                # xts holds 32 half tiles [128,512]: index kc*2 + lb
                for quarter in range(L // QL):
                    l0 = quarter * QL
                    if quarter == 0:
                        xts = xts_q0
                    else:
                        xts = (load_xt_halves(l0, 0), load_xt_halves(l0, 1))
                    # Q and K projections (transposed layout) + rope
                    for (w_sb, dst) in ((wq_sb, qt), (wk_sb, kt)):
                        for h in range(HPC):
                            for lb in range(QL // 512):
                                ps = p1ps.tile([128, 512], F32, tag="pj", bufs=4)
                                for kc in range(KCH):
                                    nc.tensor.matmul(
                                        ps[:],
                                        w_sb[:, kc * FC + h * HD : kc * FC + (h + 1) * HD],
                                        xts[lb][kc][:],
                                        start=(kc == 0),
                                        stop=(kc == KCH - 1),
                                    )
                                lsl = slice(l0 + lb * 512, l0 + (lb + 1) * 512)
                                tmp = p1.tile([128, 512], BF16, tag="tmp", bufs=4)
                                nc.scalar.copy(tmp[:], ps[:])
                                rot = p1.tile([128, 512], BF16, tag="rot", bufs=4)
                                for (a, b) in ((0, 64), (32, 96), (64, 0), (96, 32)):
                                    nc.vector.tensor_copy(
                                        rot[a : a + 32, :], tmp[b : b + 32, :]
                                    )
                                t1 = p1.tile([128, 512], BF16, tag="t1", bufs=4)
                                nc.vector.tensor_mul(t1[:], tmp[:], ctab[:, lsl])
                                t2 = p1.tile([128, 512], BF16, tag="t2", bufs=4)
                                nc.vector.tensor_mul(t2[:], rot[:], stab[:, lsl])
                                dsl = slice(
                                    h * L + l0 + lb * 512, h * L + l0 + (lb + 1) * 512
                                )
                                nc.vector.tensor_add(dst[:, dsl], t1[:], t2[:])
                    # V projection (natural layout), strided copy into va
                    for lt in range(QL // 128):
                        psv = p1ps.tile([128, FC], F32, tag="pv", bufs=3)
                        for kc in range(KCH):
                            nc.tensor.matmul(
                                psv[:],
                                xts[lt // 4][kc][:, (lt % 4) * 128 : (lt % 4 + 1) * 128],
                                wv_sb[:, kc * FC : (kc + 1) * FC],
                                start=(kc == 0),
                                stop=(kc == KCH - 1),
                            )
                        gt = quarter * (QL // 128) + lt  # global L tile 0..31
                        dst = va[:, gt * 260 : (gt + 1) * 260].rearrange(
                            "p (g j) -> p g j", g=2
                        )[:, :, 0:128]
                        nc.vector.tensor_copy(
                            dst, psv[:].rearrange("p (g j) -> p g j", g=2)
                        )

            # -------- Phase 2: attention windows + overlapped A2A/Wo ---------
            windows = [(qb, h) for qb in range(L // QB) for h in range(HPC)]
            with (
                tc.tile_pool(name="p2sb", bufs=1) as p2,
                tc.tile_pool(name="p2ps", bufs=1, space="PSUM") as p2ps,
                tc.tile_pool(name="p3sb", bufs=1) as p3,
                tc.tile_pool(name="p3ps", bufs=1, space="PSUM") as p3ps,
            ):
                pt_store = {}
                # one PSUM bank holds three packed 129-wide softmax accumulators
                av_bank = p2ps.tile([128, 512], F32, tag="avb", bufs=1)
                tr_bank = p2ps.tile([128, 512], F32, tag="tr", bufs=1)
                av_slot = [0]
                pending_epi = []

                def emit_epilogue(w, qs, base):
                    qb, h = windows[w]
                    rec = p2.tile([128, 1], F32, tag="rec", bufs=8)
                    nc.vector.reciprocal(rec[:], av_bank[:, base + 128 : base + 129])
                    osb = p2.tile([128, 128], F32, tag="osb", bufs=8)
                    nc.vector.tensor_scalar_mul(
                        osb[:], av_bank[:, base : base + 128], rec[:]
                    )
                    # PE transpose (short matmul) instead of the xbar DMA
                    # transpose (a globally serialized ~4.9us resource)
                    trs = tr_bank[:, qs * 128 : (qs + 1) * 128]
                    nc.tensor.transpose(trs, osb[:], ident_sb[:])
                    ot = p2.tile([128, 128], BF16, tag="ot", bufs=12)
                    nc.vector.tensor_copy(ot[:], trs)
                    t = qb * (QB // 128) + qs  # global q tile 0..31
                    qg = t * 128
                    s = next(
                        i for i, (q0, ql, _) in enumerate(SPLITS) if q0 <= qg < q0 + ql
                    )
                    q0, _, bq = SPLITS[s]
                    tt = (qg - q0) // 128
                    tpb = bq // 128
                    j, co = tt // tpb, (tt % tpb) * 128
                    nc.gpsimd.dma_start(
                        a2a_in[s][j * FC + h * HD : j * FC + (h + 1) * HD, co : co + 128],
                        ot[:],
                    )

                def flush_epilogues():
                    while pending_epi:
                        emit_epilogue(*pending_epi.pop(0))

                def emit_score_tile(w, kk0, tk):
                    qb, h = windows[w]
                    sc = p2ps.tile([128, 1024], F32, tag="sc", bufs=2)
                    for jt in range(tk):
                        kk = kk0 + jt
                        nc.tensor.matmul(
                            sc[:, jt * 512 : (jt + 1) * 512],
                            kt[:, h * L + kk * 128 : h * L + (kk + 1) * 128],
                            qt[:, h * L + qb * QB : h * L + (qb + 1) * QB],
                        )
                    pt = p2.tile([128, 1024], BF16, tag="pt", bufs=32)
                    nc.scalar.activation(
                        pt[:],
                        sc[:],
                        mybir.ActivationFunctionType.Exp,
                        scale=SCALE,
                    )
                    return pt

                def emit_window(w):
                    # scores+exp for window w interleaved with AV for w-1, so
                    # the PE never idles waiting on the exp pipeline
                    prev = pt_store.pop(w - 1, None)
                    if prev is not None:
                        pqb, ph = windows[w - 1]
                    pts = []
                    last_av_mm = None
                    cur = [0, 0]  # [mm cursor, current av slot base]

                    def emit_av(n):
                        nonlocal last_av_mm
                        for _ in range(n):
                            c = cur[0]
                            if c >= 128:
                                return
                            qs, kk = c // 32, c % 32
                            if kk == 0:
                                # a start-MM must never be emitted while the
                                # epilogue that reads its slot is still
                                # pending (WAR would invert into a race)
                                flush_epilogues()
                                cur[1] = (av_slot[0] % 3) * 129
                                av_slot[0] += 1
                            base = cur[1]
                            last_av_mm = nc.tensor.matmul(
                                av_bank[:, base : base + 129],
                                prev[kk // 2][
                                    :,
                                    (kk % 2) * 512
                                    + qs * 128 : (kk % 2) * 512
                                    + (qs + 1) * 128,
                                ],
                                va[:, kk * 260 + ph * 130 : kk * 260 + ph * 130 + 129],
                                start=(kk == 0),
                                stop=(kk == 31),
                                skip_group_check=True,
                            )
                            if kk == 31:
                                # defer: emitting the epilogue here makes the
                                # next group's start-matmul (emitted right
                                # after) wait ~0.65us for the epilogue's DVE
                                # reads of the shared av bank. Emitted after
                                # the next scores tile instead, the DVE reads
                                # overlap PE work and any framework-inserted
                                # ordering lands on the DVE, not the PE.
                                pending_epi.append((w - 1, qs, base))
                            cur[0] = c + 1

                    # AV chunking [12,12,8] per 32-MM accumulation group keeps
                    # group boundaries next to scores tiles
                    CH = (8,) * 16
                    kk0 = 0
                    for t, tk in enumerate(TK):
                        if w < len(windows):
                            pts.append(emit_score_tile(w, kk0, tk))
                            flush_epilogues()
                        kk0 += tk
                        if prev is not None:
                            emit_av(CH[t])
                    if prev is not None:
                        emit_av(128)  # flush any remainder
                        flush_epilogues()
                    if pts:
                        pt_store[w] = pts
                    return last_av_mm

                ats = {}

                def emit_wo_comm(s):
                    nc.gpsimd.collective_compute(
                        "AllToAll",
                        mybir.AluOpType.bypass,
                        replica_groups=[list(range(N_CORES))],
                        ins=[a2a_in[s].ap().opt()],
                        outs=[a2a_out[s].ap().opt()],
                    )

                def emit_at_load(s, after=None):
                    # deferred so the sync queue isn't head-of-line blocked
                    # on the collective while epilogue transposes queue up.
                    # 4 pieces: one dma_start runs on a single DMA engine at
                    # ~22.5GB/s (1MB = ~44us); pieces transfer in parallel
                    bq = SPLITS[s][2]
                    at = p3.tile([128, KCH * 256], BF16, tag="at", bufs=2)
                    for jp in range(8):
                        ld = nc.sync.dma_start(
                            at[:, jp * 2 * bq : (jp + 1) * 2 * bq].rearrange(
                                "p (k q) -> p k q", k=2
                            ),
                            a2a_out[s][jp * 256 : (jp + 1) * 256, :].rearrange(
                                "(k p) q -> p k q", p=128
                            ),
                        )
                        if after is not None:
                            add_dep_helper(
                                ld.ins,
                                after.ins,
                                sync=False,
                                reason="keep AT load behind the attention window",
                            )
                    ats[s] = at

                wo_out_pending = []

                def flush_wo_writes():
                    while wo_out_pending:
                        ob, r0, fb = wo_out_pending.pop(0)
                        nc.gpsimd.dma_start(
                            out_ext[r0 : r0 + 128, fb * 512 : (fb + 1) * 512], ob[:]
                        )

                def emit_wo_chunk(s, g, after=None, alt=False, defer_out=False):
                    bq = SPLITS[s][2]
                    rt, fb = g // 4, g % 4
                    if alt:
                        # tail only: scores are done, so a dead sc slot serves
                        # as the second po buffer (po itself is single-buffered)
                        po = p2ps.tile([128, 1024], F32, tag="sc", bufs=2)
                    else:
                        po = p3ps.tile([128, 512], F32, tag="po", bufs=1)
                    for kc in range(KCH):
                        mm = nc.tensor.matmul(
                            po[:, 0:512],
                            ats[s][:, kc * bq + rt * 128 : kc * bq + (rt + 1) * 128],
                            wo_sb[:, kc * D + fb * 512 : kc * D + (fb + 1) * 512],
                            start=(kc == 0),
                            stop=(kc == KCH - 1),
                            skip_group_check=True,
                        )
                        if kc == 0 and after is not None:
                            add_dep_helper(
                                mm.ins,
                                after.ins,
                                sync=False,
                                reason="keep Wo chunk behind the attention window",
                            )
                    ob = p3.tile([128, 512], F32, tag="ob", bufs=3)
                    r0 = OUT_ROW0[s] + rt * 128
                    if defer_out:
                        # tail chunks: the out-write trigger would wait on the
                        # ob copy at the gpsimd queue head and FIFO-block the
                        # final epilogue a2a writes + collective trigger.
                        # Deferred until after the comm(3) emission.
                        nc.vector.tensor_copy(ob[:], po[:, 0:512])
                        wo_out_pending.append((ob, r0, fb))
                    else:
                        # two half copies + half writes: the write of half 0
                        # overlaps the copy of half 1, and the two 128KB DMAs
                        # land on separate engines (halves the final drain);
                        # gpsimd queue: a2a writes behind have windows of slack
                        for hh in range(2):
                            nc.vector.tensor_copy(
                                ob[:, hh * 256 : (hh + 1) * 256],
                                po[:, hh * 256 : (hh + 1) * 256],
                            )
                            nc.gpsimd.dma_start(
                                out_ext[
                                    r0 : r0 + 128,
                                    fb * 512 + hh * 256 : fb * 512 + (hh + 1) * 256,
                                ],
                                ob[:, hh * 256 : (hh + 1) * 256],
                            )

                # 4-way splits: each split's epilogues finish at window 4s+4
                # and its collective launches there; the at load runs 2 windows
                # ahead of the first Wo chunk so the chunk's matmuls never
                # head-of-line-block the PE on the load DMA. One chunk per
                # window keeps the PE stream smooth; the final collective is
                # emitted before the split-2 tail chunks so it triggers the
                # moment the last epilogue lands.
                for w in range(len(windows) + 1):
                    la = emit_window(w)
                    if w == 4:
                        emit_wo_comm(0)
                    if w == 6:
                        emit_at_load(0, after=la)
                    if w == 8:
                        emit_wo_comm(1)
                        emit_wo_chunk(0, 0, after=la)
                    if w == 9:
                        emit_wo_chunk(0, 1, after=la)
                    if w == 10:
                        emit_at_load(1, after=la)
                        emit_wo_chunk(0, 2, after=la)
                    if w == 11:
                        emit_wo_chunk(0, 3, after=la)
                    if w == 12:
                        emit_wo_comm(2)
                        emit_wo_chunk(1, 0, after=la)
                    if w == 13:
                        emit_wo_chunk(1, 1, after=la)
                    if w == 14:
                        emit_at_load(2, after=la)
                        emit_wo_chunk(1, 2, after=la)
                    if w == 15:
                        emit_wo_chunk(1, 3, after=la, defer_out=True)
                        emit_wo_chunk(2, 0, after=la, defer_out=True)
                    if w == 16:
                        # split-2 tail chunks BEFORE the comm trigger: matmuls
                        # emitted after a gpsimd collective trigger wait for
                        # the collective's completion (semaphore-count
                        # inflation), which left the PE idle ~9us here. Their
                        # out-writes are deferred so the trigger stays at the
                        # head of the gpsimd queue right behind the final
                        # epilogue a2a writes (the DVE ob copy still runs
                        # inline, keeping the po chain ordered).
                        emit_wo_chunk(2, 1, defer_out=True)
                        emit_wo_chunk(2, 2, alt=True, defer_out=True)
                        emit_wo_chunk(2, 3, defer_out=True)
                        emit_wo_comm(3)
                flush_wo_writes()
                # at_load(3) emitted only after every matmul that must NOT
                # wait on it: engine semaphores are monotonic counters, so a
                # chunk emitted after this load would wait for the load's
                # completion count (observed as a 44us PE stall)
                emit_at_load(3)
                emit_wo_chunk(3, 0, alt=True)
                emit_wo_chunk(3, 1)
                emit_wo_chunk(3, 2, alt=True)
                emit_wo_chunk(3, 3)

    nc.compile()
    return nc


def _host_prep(x, Wq, Wk, Wv, Wo, sin, cos):
    xT = np.ascontiguousarray(x.T).astype(nbf16)
    wo_b = np.ascontiguousarray(Wo).astype(nbf16)
    c64 = cos.reshape(L, 64)
    s64 = sin.reshape(L, 64)
    ctab = np.ascontiguousarray(np.concatenate([c64, c64], axis=1).T).astype(nbf16)
    stab = np.ascontiguousarray(np.concatenate([-s64, s64], axis=1).T).astype(nbf16)
    ident = np.eye(128, dtype=np.float32)
    in_maps = []
    for c in range(N_CORES):
        sl = slice(c * FC, (c + 1) * FC)
        in_maps.append(
            {
                "xT": xT,
                "wq": np.ascontiguousarray(Wq[:, sl]).astype(nbf16),
                "wk": np.ascontiguousarray(Wk[:, sl]).astype(nbf16),
                "wv": np.ascontiguousarray(Wv[:, sl]).astype(nbf16),
                "wo": wo_b,
                "ctab": ctab,
                "stab": stab,
                "ident": ident,
            }
        )
    return in_maps


def kernel(x, Wq, Wk, Wv, Wo, sin, cos):
    global LAST_RESULTS
    x, Wq, Wk, Wv, Wo = (np.asarray(a, np.float32) for a in (x, Wq, Wk, Wv, Wo))
    sin, cos = np.asarray(sin, np.float32), np.asarray(cos, np.float32)

    _patch_walrus_flags()
    if TRACE:
        _install_ntff_hook()
        os.environ["BASS_TRACE"] = "1"

    if "nc" not in _CACHED:
        _CACHED["nc"] = build_nc()
    nc = _CACHED["nc"]

    in_maps = _host_prep(x, Wq, Wk, Wv, Wo, sin, cos)
    trace_cores = list(range(N_CORES)) if os.environ.get("ALL_CORES") else None
    res = run_bass_kernel_spmd(
        nc, in_maps, core_ids=list(range(N_CORES)), trace=TRACE, trace_cores=trace_cores
    )
    LAST_RESULTS = res

    out = np.empty((L, D), np.float32)
    for c in range(N_CORES):
        oc = res.results[c]["out"]
        for s in range(4):
            out[s * 1024 + c * 128 : s * 1024 + (c + 1) * 128] = oc[
                s * 128 : (s + 1) * 128
            ]
    return out



# revision 58
# speedup vs baseline: 1.0205x; 1.0152x over previous
"""Distributed multi-head attention (L=4096, D=2048, H=16, d=128) on 8 TRN2 cores.

Strategy: tensor-parallel over heads (2 heads per core) for QKV projections +
attention, then AllToAll (4x1024-row pieces, overlapped with the attention
stream) to switch to sequence-parallel for the output projection. Each core
returns 512 rows of the final output; the host reassembles.

Per-core dataflow (matmuls in bf16, f32 PSUM accumulation):
  1. QT/KT = Wq/Wk_shard.T @ x.T   ([d, L] layout, head dim on partitions)
     V     = x @ Wv_shard          ([L, f] layout, ones columns interleaved)
     RoPE applied to QT/KT in [d, L] layout with host-prepared C/S tables
     (the 1/sqrt(d) scale is folded into the exp activation).
  2. Per head: scoresT[k, q] = KT_tile.T @ QT_block -> exp on ScalarE -> PT
     bf16; out[q, d|sum] = PT.T @ [V|1] (softmax denominator rides along as a
     129th matmul column); rows normalized by its reciprocal, then transposed
     on the TensorE so the AllToAll lands in [d_concat, q] (lhsT) layout.
  3. After each A2A piece the core holds AT_s[2048, q_s] for its own output
     rows; plain chunk loads feed out = A @ Wo.

Key optimizations over the v1 baseline (trace-driven; PE measured at
~2.0 GHz sustained = the P0 power state, so matmul floors are N/2.0GHz):
- scores/exp tiles [128,1536] (3 PSUM banks, 2 buffers): 11 ScalarE exp
  instructions per window; three 129-wide softmax accumulators pack into
  one PSUM bank (av_bank) to keep the total at 8 banks.
- startup: 8-piece weight loads + 32 half-size xt tiles ordered by
  first-use; PE warm-up matmuls run on a vector-memset tile so they are
  not gated on any HBM load (engine memsets must not sit at the head of
  a DMA-issuing queue).
- 4x1024-row A2A splits: comm(s) at window 4s+4, at-load 2 windows ahead
  of the first Wo chunk (engine semaphores are monotonic counters, so a
  matmul emitted after a slow DMA on its wait-engine stalls on it), one
  Wo chunk per window from w=8.
- epilogue (reciprocal+mul on DVE) deferred past the next scores tile so
  the following AV group's start-matmul never waits on it; Wo out-writes
  ride the gpsimd queue so they cannot FIFO-block epilogue transposes on
  the sync queue ahead of the final collective.
- AV interleave chunked (12,12,8) per 32-matmul accumulation group so
  group boundaries land next to scores tiles.
"""

import os
import sys
import types

import numpy as np
import ml_dtypes

import concourse.bass as bass
import concourse.mybir as mybir
import concourse.tile as tile
from concourse import bacc
from concourse.bass_utils import run_bass_kernel_spmd
from concourse.tile_rust import add_dep_helper

BF16 = mybir.dt.bfloat16
F32 = mybir.dt.float32
nbf16 = ml_dtypes.bfloat16

N_CORES = 8
L = 4096
D = 2048
HPC = 2  # heads per core
HD = 128  # head dim
FC = HPC * HD  # 256: per-core projection width
KCH = D // 128  # 16 contraction chunks
SCALE = 1.0 / float(np.sqrt(HD))
QB = 512  # attention q block
NKK = L // 128  # 32 key tiles

# A2A splits: (q0, qlen, block_q). Output rows of core c:
#   s -> global [s*1024 + c*128, +128)
SPLITS = [(0, 1024, 128), (1024, 1024, 128), (2048, 1024, 128), (3072, 1024, 128)]
OUT_ROW0 = [0, 128, 256, 384]

# k-tiles per scores/exp tile: 16 tiles of 2 k-tiles cover a window
TK = [2] * 16

# module-level knobs (test.py pokes these)
TRACE = False
LAST_RESULTS = None
_CACHED = {}


def _patch_walrus_flags():
    from concourse import bass_utils as _bu

    if getattr(_bu, "_ldw_patched", False):
        return
    _orig = _bu.run_command

    def _patched(cmd, **kw):
        cmd = [c for c in cmd]
        return _orig(cmd, **kw)

    _bu.run_command = _patched
    _bu._ldw_patched = True


def _install_ntff_hook():
    """Enable NTFF profiling under axon (the container lacks antenv.axon_hooks)."""
    try:
        if "antenv.axon_hooks" not in sys.modules:
            mod = types.ModuleType("antenv.axon_hooks")
            _hook = [None]
            mod.set_axon_ntff_profile_hook = lambda h: _hook.__setitem__(0, h)
            mod.get_axon_ntff_profile_hook = lambda: _hook[0]
            sys.modules["antenv.axon_hooks"] = mod
            import antenv

            antenv.axon_hooks = mod
        from antenv.axon_hooks import set_axon_ntff_profile_hook
        from trn_agent_boot.trn_boot import _ntff_profile_via_ctypes

        set_axon_ntff_profile_hook(_ntff_profile_via_ctypes("/opt/axon/libaxon_pjrt.so"))
        from concourse import bass_utils

        bass_utils.upload_artifacts = lambda tmpdir: tmpdir
    except Exception:
        pass


def build_nc():
    nc = bacc.Bacc(None, target_bir_lowering=False, num_devices=N_CORES)

    xT_ext = nc.declare_dram_parameter("xT", [D, L], BF16, isOutput=False)
    wq_ext = nc.declare_dram_parameter("wq", [D, FC], BF16, isOutput=False)
    wk_ext = nc.declare_dram_parameter("wk", [D, FC], BF16, isOutput=False)
    wv_ext = nc.declare_dram_parameter("wv", [D, FC], BF16, isOutput=False)
    wo_ext = nc.declare_dram_parameter("wo", [D, D], BF16, isOutput=False)
    ctab_ext = nc.declare_dram_parameter("ctab", [128, L], BF16, isOutput=False)
    stab_ext = nc.declare_dram_parameter("stab", [128, L], BF16, isOutput=False)
    ident_ext = nc.declare_dram_parameter("ident", [128, 128], F32, isOutput=False)
    out_ext = nc.declare_dram_parameter("out", [512, D], F32, isOutput=True)

    # A2A bounces in [d_concat-block, q] layout: rows = 8 blocks x (2 heads x 128 d)
    a2a_in = [
        nc.dram_tensor(f"a2a_in{s}", [8 * FC, bq], BF16)
        for s, (_, _, bq) in enumerate(SPLITS)
    ]
    a2a_out = [
        nc.dram_tensor(f"a2a_out{s}", [8 * FC, bq], BF16)
        for s, (_, _, bq) in enumerate(SPLITS)
    ]

    sync_in = nc.dram_tensor("sync_in", [8, 64], BF16)
    sync_out = nc.dram_tensor("sync_out", [8, 64], BF16)

    with tile.TileContext(nc) as tc:
        with tc.tile_pool(name="persist", bufs=1) as persist:
            # barrier-warming collective: absorbs per-core start skew so the
            # first real AllToAll doesn't pay it
            nc.gpsimd.collective_compute(
                "AllToAll",
                mybir.AluOpType.bypass,
                replica_groups=[list(range(N_CORES))],
                ins=[sync_in.ap().opt()],
                outs=[sync_out.ap().opt()],
            )
            # persistent tiles (no DMA yet; emission order sets DMA priority)
            qt = persist.tile([128, HPC * L], BF16, tag="qt")
            kt = persist.tile([128, HPC * L], BF16, tag="kt")
            va = persist.tile([128, (L // 128) * 260], BF16, tag="va")
            wo_sb = persist.tile([128, KCH * D], BF16, tag="wo")
            warm = persist.tile([128, 8], BF16, tag="warm")
            ident_sb = persist.tile([128, 128], F32, tag="ident")
            nc.sync.dma_start(ident_sb[:], ident_ext[:])

            QL = 1024  # L columns per xT load round
            with (
                tc.tile_pool(name="p1sb", bufs=1) as p1,
                tc.tile_pool(name="p1ps", bufs=1, space="PSUM") as p1ps,
            ):
                # critical-path loads first: wq + the 16 first-half xt tiles
                # (everything the first Q projection block needs), then the
                # rest ordered by first-use time.
                ctab = p1.tile([128, L], BF16, tag="ctab")
                stab = p1.tile([128, L], BF16, tag="stab")
                wq_sb = p1.tile([128, KCH * FC], BF16, tag="wq")
                wk_sb = p1.tile([128, KCH * FC], BF16, tag="wk")
                wv_sb = p1.tile([128, KCH * FC], BF16, tag="wv")
                for jp in range(4):
                    nc.gpsimd.dma_start(
                        wq_sb[:, jp * 4 * FC : (jp + 1) * 4 * FC].rearrange(
                            "p (k f) -> p k f", k=4
                        ),
                        wq_ext[jp * 512 : (jp + 1) * 512, :].rearrange(
                            "(k p) f -> p k f", p=128
                        ),
                    )

                def load_xt_halves(l0, lb, all_sync=False):
                    ts = []
                    for kc in range(KCH):
                        xt_t = p1.tile([128, 512], BF16, tag="xt", bufs=36)
                        eng = nc.sync if all_sync else (nc.gpsimd, nc.sync)[kc % 2]
                        eng.dma_start(
                            xt_t[:],
                            xT_ext[
                                kc * 128 : (kc + 1) * 128,
                                l0 + lb * 512 : l0 + (lb + 1) * 512,
                            ],
                        )
                        ts.append(xt_t)
                    return ts

                # PE warm-up: dummy matmuls on a tiny vector-memset tile keep
                # the HAM activity window tripped during the initial DMA wait,
                # so the real matmul stream starts at full clock. The memset
                # rides the idle vector engine: an engine-op at the head of
                # the gpsimd queue would delay every startup DMA behind it.
                wtile = p1.tile([128, 128], BF16, tag="wt")
                nc.vector.memset(wtile[:], 1.0)
                wmt = p1ps.tile([128, 512], F32, tag="pj", bufs=4)
                for _ in range(110):
                    nc.tensor.matmul(wmt[0:64, 0:64], wtile[:, 0:64], wtile[:, 64:128])
                xts_q0 = (load_xt_halves(0, 0), load_xt_halves(0, 1))
                nc.gpsimd.dma_start(ctab[:], ctab_ext[:])
                nc.gpsimd.dma_start(stab[:], stab_ext[:])
                for jp in range(4):
                    nc.gpsimd.dma_start(
                        wk_sb[:, jp * 4 * FC : (jp + 1) * 4 * FC].rearrange(
                            "p (k f) -> p k f", k=4
                        ),
                        wk_ext[jp * 512 : (jp + 1) * 512, :].rearrange(
                            "(k p) f -> p k f", p=128
                        ),
                    )
                # va memset sits after the critical loads on the gpsimd queue
                # (the engine-op takes ~7us; va isn't needed until ~45us)
                nc.gpsimd.memset(va[:], 1.0)
                for jp in range(4):
                    nc.gpsimd.dma_start(
                        wv_sb[:, jp * 4 * FC : (jp + 1) * 4 * FC].rearrange(
                            "p (k f) -> p k f", k=4
                        ),
                        wv_ext[jp * 512 : (jp + 1) * 512, :].rearrange(
                            "(k p) f -> p k f", p=128
                        ),
                    )
                nc.scalar.activation(
                    warm[:], ctab[:, 0:8], mybir.ActivationFunctionType.Exp
                )
                # 4 pieces: a single-instruction DMA runs on one engine at
                # ~22.5GB/s (8MB would take ~350us); pieces transfer in parallel
                for jp in range(4):
                    nc.gpsimd.dma_start(
                        wo_sb[:, jp * 4 * D : (jp + 1) * 4 * D].rearrange(
                            "p (k f) -> p k f", k=4
                        ),
                        wo_ext[jp * 512 : (jp + 1) * 512, :].rearrange(
                            "(k p) f -> p k f", p=128
                        ),
                    )

                # ---------------- Phase 1: QKV projections + RoPE ------------
                # xts holds 32 half tiles [128,512]: index kc*2 + lb
                for quarter in range(L // QL):
                    l0 = quarter * QL
                    if quarter == 0:
                        xts = xts_q0
                    else:
                        xts = (load_xt_halves(l0, 0), load_xt_halves(l0, 1))
                    # Q and K projections (transposed layout) + rope
                    for (w_sb, dst) in ((wq_sb, qt), (wk_sb, kt)):
                        for h in range(HPC):
                            for lb in range(QL // 512):
                                ps = p1ps.tile([128, 512], F32, tag="pj", bufs=4)
                                for kc in range(KCH):
                                    nc.tensor.matmul(
                                        ps[:],
                                        w_sb[:, kc * FC + h * HD : kc * FC + (h + 1) * HD],
                                        xts[lb][kc][:],
                                        start=(kc == 0),
                                        stop=(kc == KCH - 1),
                                    )
                                lsl = slice(l0 + lb * 512, l0 + (lb + 1) * 512)
                                tmp = p1.tile([128, 512], BF16, tag="tmp", bufs=4)
                                nc.scalar.copy(tmp[:], ps[:])
                                rot = p1.tile([128, 512], BF16, tag="rot", bufs=4)
                                for (a, b) in ((0, 64), (32, 96), (64, 0), (96, 32)):
                                    nc.vector.tensor_copy(
                                        rot[a : a + 32, :], tmp[b : b + 32, :]
                                    )
                                t1 = p1.tile([128, 512], BF16, tag="t1", bufs=4)
                                nc.vector.tensor_mul(t1[:], tmp[:], ctab[:, lsl])
                                t2 = p1.tile([128, 512], BF16, tag="t2", bufs=4)
                                nc.vector.tensor_mul(t2[:], rot[:], stab[:, lsl])
                                dsl = slice(
                                    h * L + l0 + lb * 512, h * L + l0 + (lb + 1) * 512
                                )
                                nc.vector.tensor_add(dst[:, dsl], t1[:], t2[:])
                    # V projection (natural layout), strided copy into va
                    for lt in range(QL // 128):
                        psv = p1ps.tile([128, FC], F32, tag="pv", bufs=3)
                        for kc in range(KCH):
                            nc.tensor.matmul(
                                psv[:],
                                xts[lt // 4][kc][:, (lt % 4) * 128 : (lt % 4 + 1) * 128],
                                wv_sb[:, kc * FC : (kc + 1) * FC],
                                start=(kc == 0),
                                stop=(kc == KCH - 1),
                            )
                        gt = quarter * (QL // 128) + lt  # global L tile 0..31
                        dst = va[:, gt * 260 : (gt + 1) * 260].rearrange(
                            "p (g j) -> p g j", g=2
                        )[:, :, 0:128]
                        nc.vector.tensor_copy(
                            dst, psv[:].rearrange("p (g j) -> p g j", g=2)
                        )

            # -------- Phase 2: attention windows + overlapped A2A/Wo ---------
            windows = [(qb, h) for qb in range(L // QB) for h in range(HPC)]
            with (
                tc.tile_pool(name="p2sb", bufs=1) as p2,
                tc.tile_pool(name="p2ps", bufs=1, space="PSUM") as p2ps,
                tc.tile_pool(name="p3sb", bufs=1) as p3,
                tc.tile_pool(name="p3ps", bufs=1, space="PSUM") as p3ps,
            ):
                pt_store = {}
                # one PSUM bank holds three packed 129-wide softmax accumulators
                av_bank = p2ps.tile([128, 512], F32, tag="avb", bufs=1)
                tr_bank = p2ps.tile([128, 512], F32, tag="tr", bufs=1)
                av_slot = [0]
                pending_epi = []

                def emit_epilogue(w, qs, base):
                    qb, h = windows[w]
                    rec = p2.tile([128, 1], F32, tag="rec", bufs=8)
                    nc.vector.reciprocal(rec[:], av_bank[:, base + 128 : base + 129])
                    osb = p2.tile([128, 128], F32, tag="osb", bufs=8)
                    nc.vector.tensor_scalar_mul(
                        osb[:], av_bank[:, base : base + 128], rec[:]
                    )
                    # PE transpose (short matmul) instead of the xbar DMA
                    # transpose (a globally serialized ~4.9us resource)
                    trs = tr_bank[:, qs * 128 : (qs + 1) * 128]
                    nc.tensor.transpose(trs, osb[:], ident_sb[:])
                    ot = p2.tile([128, 128], BF16, tag="ot", bufs=12)
                    nc.vector.tensor_copy(ot[:], trs)
                    t = qb * (QB // 128) + qs  # global q tile 0..31
                    qg = t * 128
                    s = next(
                        i for i, (q0, ql, _) in enumerate(SPLITS) if q0 <= qg < q0 + ql
                    )
                    q0, _, bq = SPLITS[s]
                    tt = (qg - q0) // 128
                    tpb = bq // 128
                    j, co = tt // tpb, (tt % tpb) * 128
                    nc.gpsimd.dma_start(
                        a2a_in[s][j * FC + h * HD : j * FC + (h + 1) * HD, co : co + 128],
                        ot[:],
                    )

                def flush_epilogues():
                    while pending_epi:
                        emit_epilogue(*pending_epi.pop(0))

                def emit_score_tile(w, kk0, tk):
                    qb, h = windows[w]
                    sc = p2ps.tile([128, 1024], F32, tag="sc", bufs=2)
                    for jt in range(tk):
                        kk = kk0 + jt
                        nc.tensor.matmul(
                            sc[:, jt * 512 : (jt + 1) * 512],
                            kt[:, h * L + kk * 128 : h * L + (kk + 1) * 128],
                            qt[:, h * L + qb * QB : h * L + (qb + 1) * QB],
                        )
                    pt = p2.tile([128, 1024], BF16, tag="pt", bufs=32)
                    nc.scalar.activation(
                        pt[:],
                        sc[:],
                        mybir.ActivationFunctionType.Exp,
                        scale=SCALE,
                    )
                    return pt

                def emit_window(w):
                    # scores+exp for window w interleaved with AV for w-1, so
                    # the PE never idles waiting on the exp pipeline
                    prev = pt_store.pop(w - 1, None)
                    if prev is not None:
                        pqb, ph = windows[w - 1]
                    pts = []
                    last_av_mm = None
                    cur = [0, 0]  # [mm cursor, current av slot base]

                    def emit_av(n):
                        nonlocal last_av_mm
                        for _ in range(n):
                            c = cur[0]
                            if c >= 128:
                                return
                            qs, kk = c // 32, c % 32
                            if kk == 0:
                                # a start-MM must never be emitted while the
                                # epilogue that reads its slot is still
                                # pending (WAR would invert into a race)
                                flush_epilogues()
                                cur[1] = (av_slot[0] % 3) * 129
                                av_slot[0] += 1
                            base = cur[1]
                            last_av_mm = nc.tensor.matmul(
                                av_bank[:, base : base + 129],
                                prev[kk // 2][
                                    :,
                                    (kk % 2) * 512
                                    + qs * 128 : (kk % 2) * 512
                                    + (qs + 1) * 128,
                                ],
                                va[:, kk * 260 + ph * 130 : kk * 260 + ph * 130 + 129],
                                start=(kk == 0),
                                stop=(kk == 31),
                                skip_group_check=True,
                            )
                            if kk == 31:
                                # defer: emitting the epilogue here makes the
                                # next group's start-matmul (emitted right
                                # after) wait ~0.65us for the epilogue's DVE
                                # reads of the shared av bank. Emitted after
                                # the next scores tile instead, the DVE reads
                                # overlap PE work and any framework-inserted
                                # ordering lands on the DVE, not the PE.
                                pending_epi.append((w - 1, qs, base))
                            cur[0] = c + 1

                    # AV chunking [12,12,8] per 32-MM accumulation group keeps
                    # group boundaries next to scores tiles
                    CH = (8,) * 16
                    kk0 = 0
                    for t, tk in enumerate(TK):
                        if w < len(windows):
                            pts.append(emit_score_tile(w, kk0, tk))
                            flush_epilogues()
                        kk0 += tk
                        if prev is not None:
                            emit_av(CH[t])
                    if prev is not None:
                        emit_av(128)  # flush any remainder
                        flush_epilogues()
                    if pts:
                        pt_store[w] = pts
                    return last_av_mm

                ats = {}

                def emit_wo_comm(s):
                    nc.gpsimd.collective_compute(
                        "AllToAll",
                        mybir.AluOpType.bypass,
                        replica_groups=[list(range(N_CORES))],
                        ins=[a2a_in[s].ap().opt()],
                        outs=[a2a_out[s].ap().opt()],
                    )

                def emit_at_load(s, after=None):
                    # deferred so the sync queue isn't head-of-line blocked
                    # on the collective while epilogue transposes queue up.
                    # 4 pieces: one dma_start runs on a single DMA engine at
                    # ~22.5GB/s (1MB = ~44us); pieces transfer in parallel
                    bq = SPLITS[s][2]
                    at = p3.tile([128, KCH * 256], BF16, tag="at", bufs=2)
                    # 8 pieces: halves the post-collective load latency, which
                    # sits directly on the critical path for split 3 (tail DMA
                    # engines are idle, so no wave contention here)
                    for jp in range(8):
                        ld = nc.sync.dma_start(
                            at[:, jp * 2 * bq : (jp + 1) * 2 * bq].rearrange(
                                "p (k q) -> p k q", k=2
                            ),
                            a2a_out[s][jp * 256 : (jp + 1) * 256, :].rearrange(
                                "(k p) q -> p k q", p=128
                            ),
                        )
                        if after is not None:
                            add_dep_helper(
                                ld.ins,
                                after.ins,
                                sync=False,
                                reason="keep AT load behind the attention window",
                            )
                    ats[s] = at

                wo_out_pending = []

                def flush_wo_writes():
                    while wo_out_pending:
                        ob, r0, fb = wo_out_pending.pop(0)
                        nc.gpsimd.dma_start(
                            out_ext[r0 : r0 + 128, fb * 512 : (fb + 1) * 512], ob[:]
                        )

                def emit_wo_chunk(s, g, after=None, alt=False, defer_out=False):
                    bq = SPLITS[s][2]
                    rt, fb = g // 4, g % 4
                    if alt:
                        # tail only: scores are done, so a dead sc slot serves
                        # as the second po buffer (po itself is single-buffered)
                        po = p2ps.tile([128, 1024], F32, tag="sc", bufs=2)
                    else:
                        po = p3ps.tile([128, 512], F32, tag="po", bufs=1)
                    for kc in range(KCH):
                        mm = nc.tensor.matmul(
                            po[:, 0:512],
                            ats[s][:, kc * bq + rt * 128 : kc * bq + (rt + 1) * 128],
                            wo_sb[:, kc * D + fb * 512 : kc * D + (fb + 1) * 512],
                            start=(kc == 0),
                            stop=(kc == KCH - 1),
                            skip_group_check=True,
                        )
                        if kc == 0 and after is not None:
                            add_dep_helper(
                                mm.ins,
                                after.ins,
                                sync=False,
                                reason="keep Wo chunk behind the attention window",
                            )
                    ob = p3.tile([128, 512], F32, tag="ob", bufs=3)
                    r0 = OUT_ROW0[s] + rt * 128
                    if defer_out:
                        # tail chunks: the out-write trigger would wait on the
                        # ob copy at the gpsimd queue head and FIFO-block the
                        # final epilogue a2a writes + collective trigger.
                        # Deferred until after the comm(3) emission.
                        nc.vector.tensor_copy(ob[:], po[:, 0:512])
                        wo_out_pending.append((ob, r0, fb))
                    else:
                        # two half copies + half writes: the write of half 0
                        # overlaps the copy of half 1, and the two 128KB DMAs
                        # land on separate engines (halves the final drain);
                        # gpsimd queue: a2a writes behind have windows of slack
                        for hh in range(2):
                            nc.vector.tensor_copy(
                                ob[:, hh * 256 : (hh + 1) * 256],
                                po[:, hh * 256 : (hh + 1) * 256],
                            )
                            nc.gpsimd.dma_start(
                                out_ext[
                                    r0 : r0 + 128,
                                    fb * 512 + hh * 256 : fb * 512 + (hh + 1) * 256,
                                ],
                                ob[:, hh * 256 : (hh + 1) * 256],
                            )

                # 4-way splits: each split's epilogues finish at window 4s+4
                # and its collective launches there; the at load runs 2 windows
                # ahead of the first Wo chunk so the chunk's matmuls never
                # head-of-line-block the PE on the load DMA. One chunk per
                # window keeps the PE stream smooth; the final collective is
                # emitted before the split-2 tail chunks so it triggers the
                # moment the last epilogue lands.
                for w in range(len(windows) + 1):
                    la = emit_window(w)
                    if w == 4:
                        emit_wo_comm(0)
                    if w == 6:
                        emit_at_load(0, after=la)
                    if w == 8:
                        emit_wo_comm(1)
                        emit_wo_chunk(0, 0, after=la)
                    if w == 9:
                        emit_wo_chunk(0, 1, after=la)
                    if w == 10:
                        emit_at_load(1, after=la)
                        emit_wo_chunk(0, 2, after=la)
                    if w == 11:
                        emit_wo_chunk(0, 3, after=la)
                    if w == 12:
                        emit_wo_comm(2)
                        emit_wo_chunk(1, 0, after=la)
                    if w == 13:
                        emit_wo_chunk(1, 1, after=la)
                    if w == 14:
                        emit_at_load(2, after=la)
                        emit_wo_chunk(1, 2, after=la)
                    if w == 15:
                        emit_wo_chunk(1, 3, after=la, defer_out=True)
                        emit_wo_chunk(2, 0, after=la, defer_out=True)
                    if w == 16:
                        # split-2 tail chunks BEFORE the comm trigger: matmuls
                        # emitted after a gpsimd collective trigger wait for
                        # the collective's completion (semaphore-count
                        # inflation), which left the PE idle ~9us here. Their
                        # out-writes are deferred so the trigger stays at the
                        # head of the gpsimd queue right behind the final
                        # epilogue a2a writes (the DVE ob copy still runs
                        # inline, keeping the po chain ordered).
                        emit_wo_chunk(2, 1, defer_out=True)
                        emit_wo_chunk(2, 2, alt=True, defer_out=True)
                        emit_wo_chunk(2, 3, defer_out=True)
                        emit_wo_comm(3)
                flush_wo_writes()
                # at_load(3) emitted only after every matmul that must NOT
                # wait on it: engine semaphores are monotonic counters, so a
                # chunk emitted after this load would wait for the load's
                # completion count (observed as a 44us PE stall)
                emit_at_load(3)
                emit_wo_chunk(3, 0, alt=True)
                emit_wo_chunk(3, 1)
                emit_wo_chunk(3, 2, alt=True)
                emit_wo_chunk(3, 3)

    nc.compile()
    return nc


def _host_prep(x, Wq, Wk, Wv, Wo, sin, cos):
    xT = np.ascontiguousarray(x.T).astype(nbf16)
    wo_b = np.ascontiguousarray(Wo).astype(nbf16)
    c64 = cos.reshape(L, 64)
    s64 = sin.reshape(L, 64)
    ctab = np.ascontiguousarray(np.concatenate([c64, c64], axis=1).T).astype(nbf16)
    stab = np.ascontiguousarray(np.concatenate([-s64, s64], axis=1).T).astype(nbf16)
    ident = np.eye(128, dtype=np.float32)
    in_maps = []
    for c in range(N_CORES):
        sl = slice(c * FC, (c + 1) * FC)
        in_maps.append(
            {
                "xT": xT,
                "wq": np.ascontiguousarray(Wq[:, sl]).astype(nbf16),
                "wk": np.ascontiguousarray(Wk[:, sl]).astype(nbf16),
                "wv": np.ascontiguousarray(Wv[:, sl]).astype(nbf16),
                "wo": wo_b,
                "ctab": ctab,
                "stab": stab,
                "ident": ident,
            }
        )
    return in_maps


def kernel(x, Wq, Wk, Wv, Wo, sin, cos):
    global LAST_RESULTS
    x, Wq, Wk, Wv, Wo = (np.asarray(a, np.float32) for a in (x, Wq, Wk, Wv, Wo))
    sin, cos = np.asarray(sin, np.float32), np.asarray(cos, np.float32)

    _patch_walrus_flags()
    if TRACE:
        _install_ntff_hook()
        os.environ["BASS_TRACE"] = "1"

    if "nc" not in _CACHED:
        _CACHED["nc"] = build_nc()
    nc = _CACHED["nc"]

    in_maps = _host_prep(x, Wq, Wk, Wv, Wo, sin, cos)
    trace_cores = list(range(N_CORES)) if os.environ.get("ALL_CORES") else None
    res = run_bass_kernel_spmd(
        nc, in_maps, core_ids=list(range(N_CORES)), trace=TRACE, trace_cores=trace_cores
    )
    LAST_RESULTS = res

    out = np.empty((L, D), np.float32)
    for c in range(N_CORES):
        oc = res.results[c]["out"]
        for s in range(4):
            out[s * 1024 + c * 128 : s * 1024 + (c + 1) * 128] = oc[
                s * 128 : (s + 1) * 128
            ]
    return out

